# revision 3
# baseline (speedup 1.0000x reference)
"""Trainium2 Bass kernel for nn_Encoder_46943992545741 (gnn_message_passing).

Math (see reference):
  uw = cumsum(u_weight, 0); vw = cumsum(v_weight, 0)
  tmp_u[r,n,h] = u_feat[n,:] @ uw[r]     tmp_v[r,m,h] = v_feat[m,:] @ vw[r]
  sn[r,n,m] = support[r,n,m] * rsqrt(rowsum[r,n]) * rsqrt(colsum[r,m])
  ZU[n,h] = sum_r sum_m sn[r,n,m] * tmp_v[r,m,h]
  ZV[m,h] = sum_r sum_n sn[r,n,m] * tmp_u[r,n,h]
  z_u = relu(ZU[u] + bias); z_v = relu(ZV[v] + bias)

Distribution: shard the user axis (Nu) across 8 cores; no collectives.  The
host folds the symmetric normalization directly into support (sn, scaled by
2**17 for fp8 range) so each core's work is fully independent: the V side
contracts its 512 local rows against tmp_u (partial ZV, summed across cores on
the host), the U side contracts all 4096 columns of its row-shard against
tmp_v (exact local ZU rows, concatenated on the host).

Both orientations of the sn shard ship as fp8 e4m3 (2 x 10.5MB per core - a
quarter of the f32 volume) in a pre-swizzled per-partition-contiguous layout
(16KB contiguous per partition per relation => line-rate DMA).  tmp_u / tmp_v
are computed on-device in bf16 and cast to fp8; the streaming contractions run
as DoubleRow fp8 matmuls (K=256 per pass, 2 MACs/cell/cycle).  Each side
accumulates all 5 relations into a single PSUM region and drains once.
quantization error (measured against the f64 oracle): ~4.6e-3 max-scale-rel.
"""

import numpy as np
import ml_dtypes
from contextlib import ExitStack

import concourse.bass as bass
import concourse.bacc as bacc
import concourse.mybir as mybir
import concourse.tile as tile
from concourse.bass_utils import run_bass_kernel_spmd

BF16 = mybir.dt.bfloat16
FP8 = mybir.dt.float8e4
F32 = mybir.dt.float32
DOUBLE_ROW = mybir.MatmulPerfMode.DoubleRow

NCORES = 8
NU = 4096
NV = 4096
D = 256
H = 64
R = 5
SN_SCALE = 131072.0  # 2**17: sn values (~5e-4 max) into e4m3's sweet range


def build_program(ncores=NCORES, nu=NU, nv=NV, d=D, h=H, r=R, repeat=1):
    nsh = nu // ncores           # rows per core (512)
    nbc = nsh // 128             # n strips per relation (4)
    mbc = nv // 128              # m blocks of 128 (32)
    dbc = d // 128               # contraction blocks for feature matmuls (2)
    vhalf = nv // 2              # V-side psum half width (2048)

    nc = bacc.Bacc()
    sn_n = nc.dram_tensor("sn_n", [r, 128, nbc, nv], FP8, kind="ExternalInput")
    sn_t = nc.dram_tensor("sn_t", [r, 128, mbc, nsh], FP8, kind="ExternalInput")
    ufT = nc.dram_tensor("ufT", [dbc, 128, nsh], BF16, kind="ExternalInput")
    vfT = nc.dram_tensor("vfT", [dbc, 128, nv], BF16, kind="ExternalInput")
    uwt = nc.dram_tensor("uwt", [dbc, 128, r * h], BF16, kind="ExternalInput")
    vwt = nc.dram_tensor("vwt", [dbc, 128, r * h], BF16, kind="ExternalInput")
    zu_o = nc.dram_tensor("zu_o", [h, nsh], F32, kind="ExternalOutput")
    zv_o = nc.dram_tensor("zv_o", [h, nv], F32, kind="ExternalOutput")

    with tile.TileContext(nc) as tc, ExitStack() as ctx:
        wpool = ctx.enter_context(tc.tile_pool(name="weights", bufs=1))
        tmp = ctx.enter_context(tc.tile_pool(name="tmp", bufs=1))
        stream_pool = ctx.enter_context(tc.tile_pool(name="stream", bufs=6))
        zvs_pool = ctx.enter_context(tc.tile_pool(name="zvs", bufs=2))
        zus_pool = ctx.enter_context(tc.tile_pool(name="zus", bufs=2))

        ufT_sb = wpool.tile([128, dbc, nsh], BF16)
        vfT_sb = wpool.tile([128, dbc, nv], BF16)
        uw_sb = wpool.tile([128, dbc, r * h], BF16)
        vw_sb = wpool.tile([128, dbc, r * h], BF16)
        tmpu8 = tmp.tile([128, r, nbc, h], FP8)
        tmpv8 = tmp.tile([128, r, mbc, h], FP8)
        for db in range(dbc):
            nc.sync.dma_start(ufT_sb[:, db, :], ufT[db])
            nc.sync.dma_start(uw_sb[:, db, :], uwt[db])
            nc.sync.dma_start(vw_sb[:, db, :], vwt[db])
            nc.sync.dma_start(vfT_sb[:, db, :], vfT[db])

        for _rep in range(repeat):
            # ---- phase 0: tmp_u / tmp_v (feature x cumsum-weight matmuls),
            # psum f32 -> fp8 tiles (DVE / ACT split the casts) ----
            with tc.tile_pool(name="psum0", bufs=4, space="PSUM") as psum0:
                for nb in range(nbc):
                    p0 = psum0.tile([128, r * h], F32)
                    for db in range(dbc):
                        nc.tensor.matmul(
                            p0[:], ufT_sb[:, db, nb * 128:(nb + 1) * 128],
                            uw_sb[:, db, :], start=(db == 0), stop=(db == dbc - 1))
                    nc.vector.tensor_copy(
                        tmpu8[:, :, nb, :],
                        p0[:].rearrange("p (r h) -> p r h", r=r))
                for mb in range(mbc):
                    p0 = psum0.tile([128, r * h], F32)
                    for db in range(dbc):
                        nc.tensor.matmul(
                            p0[:], vfT_sb[:, db, mb * 128:(mb + 1) * 128],
                            vw_sb[:, db, :], start=(db == 0), stop=(db == dbc - 1))
                    eng = nc.vector.tensor_copy if mb % 2 == 0 else nc.scalar.copy
                    eng(tmpv8[:, :, mb, :],
                        p0[:].rearrange("p (r h) -> p r h", r=r))

            # ---- V side: ZV_partial[h, m] = sum_r sum_{n local} sn*tmp_u ----
            # One accumulation per psum half across all relations; DoubleRow
            # contracts n in pairs of 128-strips.
            with tc.tile_pool(name="psumV", bufs=1, space="PSUM") as psumV:
                pv = [psumV.tile([h, vhalf], F32, name=f"pv{i}", tag=f"pv{i}",
                                 bufs=1) for i in range(2)]
                snt = []
                for rr in range(r):
                    st = stream_pool.tile([128, nbc, nv], FP8, name="stm",
                                          tag="stm")
                    nc.sync.dma_start(st[:], sn_n[rr])
                    snt.append(st)
                # fp8 moving operand max is 1024 elements = 2x512 in
                # DoubleRow, so emit 512-wide chunks.
                qw = 512
                for rr in range(r):
                    for bb in range(nbc // 2):
                        for q in range(nv // qw):
                            half, qq = divmod(q, vhalf // qw)
                            nc.tensor.matmul(
                                pv[half][:, qq * qw:(qq + 1) * qw],
                                tmpu8[:, rr, 2 * bb:2 * bb + 2, :],
                                snt[rr][:, 2 * bb:2 * bb + 2,
                                        q * qw:(q + 1) * qw],
                                start=(rr == 0 and bb == 0),
                                stop=(rr == r - 1 and bb == nbc // 2 - 1),
                                perf_mode=DOUBLE_ROW)
                for half, eng in ((0, nc.vector.tensor_copy), (1, nc.scalar.copy)):
                    stg = zvs_pool.tile([h, vhalf], F32, name="stg", tag="stg")
                    eng(stg[:], pv[half][:])
                    nc.scalar.dma_start(
                        zv_o[:, half * vhalf:(half + 1) * vhalf], stg[:])

            # ---- U side: ZU[h, n local] = sum_r sum_m sn^T*tmp_v ----
            with tc.tile_pool(name="psumU", bufs=1, space="PSUM") as psumU:
                pu = psumU.tile([h, nsh], F32, name="pu", tag="pu", bufs=1)
                stt = []
                for rr in range(r):
                    st = stream_pool.tile([128, mbc, nsh], FP8, name="stm",
                                          tag="stm")
                    nc.sync.dma_start(st[:], sn_t[rr])
                    stt.append(st)
                for rr in range(r):
                    for jj in range(mbc // 2):
                        nc.tensor.matmul(
                            pu[:], tmpv8[:, rr, 2 * jj:2 * jj + 2, :],
                            stt[rr][:, 2 * jj:2 * jj + 2, :],
                            start=(rr == 0 and jj == 0),
                            stop=(rr == r - 1 and jj == mbc // 2 - 1),
                            perf_mode=DOUBLE_ROW)
                zs = zus_pool.tile([h, nsh], F32, name="zs", tag="zs")
                nc.vector.tensor_copy(zs[:], pu[:])
                nc.scalar.dma_start(zu_o[:], zs[:])
    nc.finalize()
    return nc


def prep_inputs(u_feat, v_feat, support, u_weight, v_weight,
                ncores=NCORES):
    """Host-side sharding / layout prep.  Returns per-core input dicts."""
    bf = ml_dtypes.bfloat16
    f8 = ml_dtypes.float8_e4m3
    r, nu, nv = support.shape
    d, h = u_weight.shape[1], u_weight.shape[2]
    dbc = d // 128
    nsh = nu // ncores
    nbc = nsh // 128
    mbc = nv // 128

    sup = support.astype(np.float32)
    col = sup.sum(axis=1)                      # [r, nv]
    row = sup.sum(axis=2)                      # [r, nu]
    with np.errstate(divide="ignore"):
        rinv = np.where(col > 0, 1.0 / np.sqrt(col), 0.0)
        cinv = np.where(row > 0, 1.0 / np.sqrt(row), 0.0)
    sn8 = (sup * (cinv[:, :, None] * (SN_SCALE * rinv[:, None, :]))).astype(f8)

    uw = np.cumsum(u_weight.astype(np.float32), axis=0)
    vw = np.cumsum(v_weight.astype(np.float32), axis=0)

    def wt(w):  # [r, d, h] -> [dbc, 128, r*h]
        return np.ascontiguousarray(
            w.reshape(r, dbc, 128, h).transpose(1, 2, 0, 3)
            .reshape(dbc, 128, r * h)).astype(bf)

    ufT = np.ascontiguousarray(u_feat.T).astype(bf)       # [d, nu]
    vfT = np.ascontiguousarray(v_feat.T).astype(bf)       # [d, nv]
    vfT_d = vfT.reshape(dbc, 128, nv)
    uwt_d, vwt_d = wt(uw), wt(vw)

    sn8T = sn8.transpose(0, 2, 1)                          # [r, nv, nu] view
    in_maps = []
    for c in range(ncores):
        sl = slice(c * nsh, (c + 1) * nsh)
        # natural: [rr, p, g, m] = sn[rr, c*nsh + g*128 + p, m]
        nat = np.ascontiguousarray(
            sn8[:, sl, :].reshape(r, nbc, 128, nv).transpose(0, 2, 1, 3))
        # transposed: [rr, p, j, n] = sn[rr, c*nsh + n, j*128 + p]
        tr = np.ascontiguousarray(
            sn8T[:, :, sl].reshape(r, mbc, 128, nsh).transpose(0, 2, 1, 3))
        in_maps.append({
            "sn_n": nat,
            "sn_t": tr,
            "ufT": np.ascontiguousarray(ufT[:, sl]).reshape(dbc, 128, nsh),
            "vfT": vfT_d,
            "uwt": uwt_d,
            "vwt": vwt_d,
        })
    return in_maps


def postprocess(results, u, v, u_bias, ncores=NCORES):
    """Combine per-core partials into (relu(z_u), relu(z_v))."""
    ZU = np.concatenate([results[c]["zu_o"] for c in range(ncores)], axis=1)
    ZV = sum(results[c]["zv_o"].astype(np.float64) for c in range(ncores))
    ZU = ZU.astype(np.float64).T / SN_SCALE    # [nu, h]
    ZV = ZV.T / SN_SCALE                       # [nv, h]
    bias = np.asarray(u_bias, np.float64)
    zu = np.maximum(ZU[np.asarray(u)] + bias, 0.0).astype(np.float32)
    zv = np.maximum(ZV[np.asarray(v)] + bias, 0.0).astype(np.float32)
    return zu, zv


_PROGRAM = None


def kernel(u_feat, v_feat, u, v, support, u_weight, v_weight, u_bias,
           **run_kwargs):
    global _PROGRAM
    u_feat = np.asarray(u_feat, np.float32)
    v_feat = np.asarray(v_feat, np.float32)
    support = np.asarray(support, np.float32)
    u_weight = np.asarray(u_weight, np.float32)
    v_weight = np.asarray(v_weight, np.float32)
    u = np.asarray(u)
    v = np.asarray(v)

    if _PROGRAM is None:
        _PROGRAM = build_program()
    in_maps = prep_inputs(u_feat, v_feat, support, u_weight, v_weight)
    res = run_bass_kernel_spmd(
        _PROGRAM, in_maps, core_ids=list(range(NCORES)), **run_kwargs)
    return postprocess(res.results, u, v, np.asarray(u_bias, np.float32))


# revision 6
# speedup vs baseline: 13.9950x; 13.9950x over previous
"""Trainium2 Bass kernel for nn_Encoder_46943992545741 (gnn_message_passing).

Math (see reference):
  uw = cumsum(u_weight, 0); vw = cumsum(v_weight, 0)
  tmp_u[r,n,h] = u_feat[n,:] @ uw[r]     tmp_v[r,m,h] = v_feat[m,:] @ vw[r]
  sn[r,n,m] = support[r,n,m] * rsqrt(rowsum[r,n]) * rsqrt(colsum[r,m])
  ZU[n,h] = sum_r sum_m sn[r,n,m] * tmp_v[r,m,h]
  ZV[m,h] = sum_r sum_n sn[r,n,m] * tmp_u[r,n,h]
  z_u = relu(ZU[u] + bias); z_v = relu(ZV[v] + bias)

Distribution: shard the user axis (Nu) across 8 cores; no collectives.  The
host folds the symmetric normalization directly into support (sn, scaled by
2**17 for fp8 range) so each core's work is fully independent: the V side
contracts its 512 local rows against tmp_u (partial ZV, summed across cores on
the host), the U side contracts all 4096 columns of its row-shard against
tmp_v (exact local ZU rows, concatenated on the host).

Both orientations of the sn shard ship as fp8 e4m3 (2 x 10.5MB per core - a
quarter of the f32 volume) in a pre-swizzled per-partition-contiguous layout
(16KB contiguous per partition per relation => line-rate DMA).  tmp_u / tmp_v
are computed on-device in bf16 and cast to fp8; the streaming contractions run
as DoubleRow fp8 matmuls (K=256 per pass, 2 MACs/cell/cycle).  Each side
accumulates all 5 relations into a single PSUM region and drains once.
quantization error (measured against the f64 oracle): ~4.6e-3 max-scale-rel.
"""

import numpy as np
import ml_dtypes
from contextlib import ExitStack

import concourse.bass as bass
import concourse.bacc as bacc
import concourse.mybir as mybir
import concourse.tile as tile
from concourse.bass_utils import run_bass_kernel_spmd

BF16 = mybir.dt.bfloat16
FP8 = mybir.dt.float8e4
F32 = mybir.dt.float32
DOUBLE_ROW = mybir.MatmulPerfMode.DoubleRow

NCORES = 8
NU = 4096
NV = 4096
D = 256
H = 64
R = 5
SN_SCALE = 131072.0  # 2**17: sn values (~5e-4 max) into e4m3's sweet range
W_SCALE = 8.0        # 2**3: cumsum-weight values (~0.08 rms) into e4m3 range


def build_program(ncores=NCORES, nu=NU, nv=NV, d=D, h=H, r=R, repeat=1):
    nsh = nu // ncores           # rows per core (512)
    nbc = nsh // 128             # n strips per relation (4)
    mbc = nv // 128              # m blocks of 128 (32)
    dbc = d // 128               # contraction blocks for feature matmuls (2)
    vhalf = nv // 2              # V-side psum half width (2048)

    nc = bacc.Bacc()
    sn_n = nc.dram_tensor("sn_n", [r, 128, nbc, nv], FP8, kind="ExternalInput")
    sn_t = nc.dram_tensor("sn_t", [r, 128, mbc, nsh], FP8, kind="ExternalInput")
    ufT = nc.dram_tensor("ufT", [128, dbc, nsh], FP8, kind="ExternalInput")
    vfT = nc.dram_tensor("vfT", [128, dbc, nv], FP8, kind="ExternalInput")
    uwt = nc.dram_tensor("uwt", [128, dbc, r * h], FP8, kind="ExternalInput")
    vwt = nc.dram_tensor("vwt", [128, dbc, r * h], FP8, kind="ExternalInput")
    zu_o = nc.dram_tensor("zu_o", [h, nsh], F32, kind="ExternalOutput")
    zv_o = nc.dram_tensor("zv_o", [h, nv], BF16, kind="ExternalOutput")

    with tile.TileContext(nc) as tc, ExitStack() as ctx:
        wpool = ctx.enter_context(tc.tile_pool(name="weights", bufs=1))
        tmp = ctx.enter_context(tc.tile_pool(name="tmp", bufs=1))
        stream_pool = ctx.enter_context(tc.tile_pool(name="stream", bufs=6))
        zvs_pool = ctx.enter_context(tc.tile_pool(name="zvs", bufs=2))
        zus_pool = ctx.enter_context(tc.tile_pool(name="zus", bufs=2))

        ufT_sb = wpool.tile([128, dbc, nsh], FP8)
        vfT_sb = wpool.tile([128, dbc, nv], FP8)
        uw_sb = wpool.tile([128, dbc, r * h], FP8)
        vw_sb = wpool.tile([128, dbc, r * h], FP8)
        tmpu8 = tmp.tile([128, r, nbc, h], FP8)
        tmpv8 = tmp.tile([128, r, mbc, h], FP8)
        nc.sync.dma_start(ufT_sb[:], ufT[:])
        nc.sync.dma_start(uw_sb[:], uwt[:])
        nc.sync.dma_start(vw_sb[:], vwt[:])
        nc.sync.dma_start(vfT_sb[:], vfT[:])

        for _rep in range(repeat):
            # ---- phase 0: tmp_u / tmp_v (feature x cumsum-weight matmuls),
            # DoubleRow over the d=2x128 contraction; psum f32 -> fp8 tiles
            # (DVE / ACT split the casts) ----
            with tc.tile_pool(name="psum0", bufs=4, space="PSUM") as psum0:
                for nb in range(nbc):
                    p0 = psum0.tile([128, r * h], F32)
                    nc.tensor.matmul(
                        p0[:], ufT_sb[:, :, nb * 128:(nb + 1) * 128],
                        uw_sb[:], start=True, stop=True, perf_mode=DOUBLE_ROW)
                    nc.vector.tensor_copy(
                        tmpu8[:, :, nb, :],
                        p0[:].rearrange("p (r h) -> p r h", r=r))
                for mb in range(mbc):
                    p0 = psum0.tile([128, r * h], F32)
                    nc.tensor.matmul(
                        p0[:], vfT_sb[:, :, mb * 128:(mb + 1) * 128],
                        vw_sb[:], start=True, stop=True, perf_mode=DOUBLE_ROW)
                    eng = nc.vector.tensor_copy if mb % 2 == 0 else nc.scalar.copy
                    eng(tmpv8[:, :, mb, :],
                        p0[:].rearrange("p (r h) -> p r h", r=r))

            # ---- V side: ZV_partial[h, m] = sum_r sum_{n local} sn*tmp_u ----
            # One accumulation per psum half across all relations; DoubleRow
            # contracts n in pairs of 128-strips.
            with tc.tile_pool(name="psumV", bufs=1, space="PSUM") as psumV:
                pv = [psumV.tile([h, vhalf], F32, name=f"pv{i}", tag=f"pv{i}",
                                 bufs=1) for i in range(2)]
                snt = []
                for rr in range(r):
                    st = stream_pool.tile([128, nbc, nv], FP8, name="stm",
                                          tag="stm")
                    nc.sync.dma_start(st[:], sn_n[rr])
                    snt.append(st)
                # fp8 moving operand max is 1024 elements = 2x512 in
                # DoubleRow, so emit 512-wide chunks.
                qw = 512
                for rr in range(r):
                    for bb in range(nbc // 2):
                        for q in range(nv // qw):
                            half, qq = divmod(q, vhalf // qw)
                            nc.tensor.matmul(
                                pv[half][:, qq * qw:(qq + 1) * qw],
                                tmpu8[:, rr, 2 * bb:2 * bb + 2, :],
                                snt[rr][:, 2 * bb:2 * bb + 2,
                                        q * qw:(q + 1) * qw],
                                start=(rr == 0 and bb == 0),
                                stop=(rr == r - 1 and bb == nbc // 2 - 1),
                                perf_mode=DOUBLE_ROW)
                for half, eng in ((0, nc.vector.tensor_copy), (1, nc.scalar.copy)):
                    stg = zvs_pool.tile([h, vhalf], F32, name="stg", tag="stg")
                    eng(stg[:], pv[half][:])
                    nc.scalar.dma_start(
                        zv_o[:, half * vhalf:(half + 1) * vhalf], stg[:])

            # ---- U side: ZU[h, n local] = sum_r sum_m sn^T*tmp_v ----
            with tc.tile_pool(name="psumU", bufs=1, space="PSUM") as psumU:
                pu = psumU.tile([h, nsh], F32, name="pu", tag="pu", bufs=1)
                stt = []
                for rr in range(r):
                    st = stream_pool.tile([128, mbc, nsh], FP8, name="stm",
                                          tag="stm")
                    nc.sync.dma_start(st[:], sn_t[rr])
                    stt.append(st)
                for rr in range(r):
                    for jj in range(mbc // 2):
                        nc.tensor.matmul(
                            pu[:], tmpv8[:, rr, 2 * jj:2 * jj + 2, :],
                            stt[rr][:, 2 * jj:2 * jj + 2, :],
                            start=(rr == 0 and jj == 0),
                            stop=(rr == r - 1 and jj == mbc // 2 - 1),
                            perf_mode=DOUBLE_ROW)
                zs = zus_pool.tile([h, nsh], F32, name="zs", tag="zs")
                nc.vector.tensor_copy(zs[:], pu[:])
                nc.scalar.dma_start(zu_o[:], zs[:])
    nc.finalize()
    return nc


def prep_inputs(u_feat, v_feat, support, u_weight, v_weight,
                ncores=NCORES):
    """Host-side sharding / layout prep.  Returns per-core input dicts."""
    bf = ml_dtypes.bfloat16
    f8 = ml_dtypes.float8_e4m3
    r, nu, nv = support.shape
    d, h = u_weight.shape[1], u_weight.shape[2]
    dbc = d // 128
    nsh = nu // ncores
    nbc = nsh // 128
    mbc = nv // 128

    sup = support.astype(np.float32)
    col = sup.sum(axis=1)                      # [r, nv]
    row = sup.sum(axis=2)                      # [r, nu]
    with np.errstate(divide="ignore"):
        rinv = np.where(col > 0, 1.0 / np.sqrt(col), 0.0)
        cinv = np.where(row > 0, 1.0 / np.sqrt(row), 0.0)
    sn8 = (sup * (cinv[:, :, None] * (SN_SCALE * rinv[:, None, :]))).astype(f8)

    uw = np.cumsum(u_weight.astype(np.float32), axis=0)
    vw = np.cumsum(v_weight.astype(np.float32), axis=0)

    def wt(w):  # [r, d, h] -> [dbc, 128, r*h]
        return np.ascontiguousarray(
            w.reshape(r, dbc, 128, h).transpose(1, 2, 0, 3)
            .reshape(dbc, 128, r * h)).astype(bf)

    ufT = np.ascontiguousarray(u_feat.T).astype(bf)       # [d, nu]
    vfT = np.ascontiguousarray(v_feat.T).astype(bf)       # [d, nv]
    vfT_d = vfT.reshape(dbc, 128, nv)
    uwt_d, vwt_d = wt(uw), wt(vw)

    sn8T = sn8.transpose(0, 2, 1)                          # [r, nv, nu] view
    in_maps = []
    for c in range(ncores):
        sl = slice(c * nsh, (c + 1) * nsh)
        # natural: [rr, p, g, m] = sn[rr, c*nsh + g*128 + p, m]
        nat = np.ascontiguousarray(
            sn8[:, sl, :].reshape(r, nbc, 128, nv).transpose(0, 2, 1, 3))
        # transposed: [rr, p, j, n] = sn[rr, c*nsh + n, j*128 + p]
        tr = np.ascontiguousarray(
            sn8T[:, :, sl].reshape(r, mbc, 128, nsh).transpose(0, 2, 1, 3))
        in_maps.append({
            "sn_n": nat,
            "sn_t": tr,
            "ufT": np.ascontiguousarray(ufT[:, sl]).reshape(dbc, 128, nsh),
            "vfT": vfT_d,
            "uwt": uwt_d,
            "vwt": vwt_d,
        })
    return in_maps


def postprocess(results, u, v, u_bias, ncores=NCORES):
    """Combine per-core partials into (relu(z_u), relu(z_v))."""
    ZU = np.concatenate([results[c]["zu_o"] for c in range(ncores)], axis=1)
    ZV = sum(results[c]["zv_o"].astype(np.float64) for c in range(ncores))
    ZU = ZU.astype(np.float64).T / SN_SCALE    # [nu, h]
    ZV = ZV.T / SN_SCALE                       # [nv, h]
    bias = np.asarray(u_bias, np.float64)
    zu = np.maximum(ZU[np.asarray(u)] + bias, 0.0).astype(np.float32)
    zv = np.maximum(ZV[np.asarray(v)] + bias, 0.0).astype(np.float32)
    return zu, zv


_PROGRAM = None


def kernel(u_feat, v_feat, u, v, support, u_weight, v_weight, u_bias,
           **run_kwargs):
    global _PROGRAM
    u_feat = np.asarray(u_feat, np.float32)
    v_feat = np.asarray(v_feat, np.float32)
    support = np.asarray(support, np.float32)
    u_weight = np.asarray(u_weight, np.float32)
    v_weight = np.asarray(v_weight, np.float32)
    u = np.asarray(u)
    v = np.asarray(v)

    if _PROGRAM is None:
        _PROGRAM = build_program()
    in_maps = prep_inputs(u_feat, v_feat, support, u_weight, v_weight)
    res = run_bass_kernel_spmd(
        _PROGRAM, in_maps, core_ids=list(range(NCORES)), **run_kwargs)
    return postprocess(res.results, u, v, np.asarray(u_bias, np.float32))
